# revision 27
# baseline (speedup 1.0000x reference)
"""Trainium2 Bass kernel for a pre-LN transformer block (B=2, T=2048, D=1024,
NH=16, HD=64, DFF=4096) on 8 NeuronCores.

Sharding: 4 cores per batch. Core j of a batch owns query tiles
{j, 4+j, 8+j, 12+j} (one per "slot" s=0..3), so every core does identical
causal-attention work: slot s attends to key tiles 0..4s+3 (40 score tiles
per head instead of 64). The host permutes each batch's tokens so the core's
own tile of group g sits at group position 4g+3; the key set covered by slot
s is unchanged, and causality within the last 4 key tiles of each slot is
applied via a small per-core data mask.

Precision: K/V/Q/proj/fc1/fc2 matmuls run in fp8e4m3 with DoubleRow perf
mode (two 128-deep contraction tiles per instruction). Weights are
pre-scaled by a power of two into fp8 normal range; the inverse scale folds
into the PSUM-evacuation copies. Scores and att@V stay bf16; fp32 PSUM
accumulation everywhere. ln1 runs channel-major via PE ones-matmul stats and
a rank-1 PSUM broadcast; ln2 token-major via bn_stats. The residual stream
is carried as 16*x so the proj/fc2 weight scale (16) cancels into a single
add per chunk.
"""

import sys

for _p in ("/opt/trn_rl_repo", "/root/.axon_site/_ro/trn_rl_repo"):
    if _p not in sys.path:
        sys.path.insert(0, _p)

import numpy as np
import ml_dtypes

import concourse.bass as bass
import concourse.tile as tile
from concourse import bacc, mybir
from concourse.bass_utils import run_bass_kernel_spmd

B = 2
T = 2048
D = 1024
NH = 16
HD = 64
DFF = 4 * D
EPS = 1e-5
P = 128
KO = D // P            # 8 contraction tiles over D
KP = KO // 2           # 4 DoubleRow pairs over D
N_CORES = 8
CPB = N_CORES // B     # cores per batch
TC = T // CPB          # 512 own tokens per core
NS = 4                 # query slots per core
NKT = T // P           # 16 key tiles
NFT = DFF // P         # 32 dff tiles
FP = NFT // 2          # 16 DoubleRow pairs over DFF
FC = 512
NCH = T // FC          # 4 token chunks for ln1
CS = [4, 8, 12, 16]    # key tiles per slot
TK = sum(CS)           # 40 score tiles per head

f32 = mybir.dt.float32
f32r = mybir.dt.float32r
bf16 = mybir.dt.bfloat16
fp8 = mybir.dt.float8e4
AF = mybir.ActivationFunctionType
ALU = mybir.AluOpType
DR = mybir.MatmulPerfMode.DoubleRow

_CACHE = {}


def build_nc():
    nc = bacc.Bacc("TRN2", target_bir_lowering=False)

    io = {}
    d = nc.declare_dram_parameter
    io["xbT"] = d("xbT", [D, T], bf16, isOutput=False)          # permuted, ch-major
    io["x_own16"] = d("x_own16", [TC, D], f32, isOutput=False)  # 16*(x_own+bvproj)
    io["w_k8"] = d("w_k8", [P, KO, KP, 2, P], fp8, isOutput=False)
    io["w_q8"] = d("w_q8", [P, KO, KP, 2, P], fp8, isOutput=False)
    io["w_v8"] = d("w_v8", [P, KP, 2, D], fp8, isOutput=False)
    io["w_p8"] = d("w_p8", [P, KP, 2, D], fp8, isOutput=False)
    io["fc1_w8"] = d("fc1_w8", [P, NFT, KP, 2, P], fp8, isOutput=False)
    io["fc2_w8"] = d("fc2_w8", [P, FP, 2, D], fp8, isOutput=False)
    io["b_k"] = d("b_k", [P, KO], f32, isOutput=False)
    io["b_q"] = d("b_q", [P, KO], f32, isOutput=False)
    io["fc1_b"] = d("fc1_b", [P, NFT], f32, isOutput=False)
    io["b2_rep16"] = d("b2_rep16", [P, D], f32, isOutput=False)
    io["mask"] = d("mask", [P, NS, 4, P], bf16, isOutput=False)
    io["ident"] = d("ident", [P, P], f32r, isOutput=False)
    io["out"] = d("out", [TC, D], f32, isOutput=True)

    with tile.TileContext(nc) as tc:
        _emit(nc, tc, io)
    nc.compile()
    return nc


def _emit(nc, tc, io):
    from contextlib import ExitStack

    with ExitStack() as ctx:
        singles = ctx.enter_context(tc.tile_pool(name="singles", bufs=1))

        ones_invD = singles.tile([P, 1], bf16)
        nc.vector.memset(ones_invD, 1.0 / D)
        eps1 = singles.tile([1, 1], f32)
        nc.vector.memset(eps1, EPS)
        epsT = singles.tile([P, 1], f32)
        nc.vector.memset(epsT, EPS * 256.0)          # ln2 runs on 16*x
        ident = singles.tile([P, P], f32r)
        identb = singles.tile([P, P], bf16)
        nc.vector.tensor_copy(out=identb, in_=ident)

        # ---- tiles that live into phases D/E (must be below pMain on the
        # pool stack, so allocated up front) ----
        pLate_cm = tc.tile_pool(name="pLate", bufs=1)
        pLate = pLate_cm.__enter__()
        attn_out = pLate.tile([P, NS, D], bf16)
        attn_outT = pLate.tile([P, KO, TC], fp8)
        X2 = pLate.tile([P, NS, D], f32)
        xn2T = pLate.tile([P, KO, TC], fp8)
        w_p8 = pLate.tile([P, KP, 2, D], fp8)

        # ---- persistent big tiles (live through phase C) ----
        pMain_cm = tc.tile_pool(name="pMain", bufs=1)
        pMain = pMain_cm.__enter__()
        kT = pMain.tile([P, KO, T], bf16)
        v_aug = pMain.tile([P, NKT, NH, HD + 1], bf16)
        qT = pMain.tile([P, KO, TC], bf16)

        # ---------- Phase A: ln1 channel-major, V interleaved ----------
        pA_cm = tc.tile_pool(name="pA", bufs=1)
        pA = pA_cm.__enter__()
        xn8 = pA.tile([P, KO, T], fp8)
        w_v8 = pA.tile([P, KP, 2, D], fp8)
        w_k8 = pA.tile([P, KO, KP, 2, P], fp8)
        w_q8 = pA.tile([P, KO, KP, 2, P], fp8)
        b_k8 = pA.tile([P, KO], f32)
        b_q8 = pA.tile([P, KO], f32)
        nc.vector.memset(v_aug[:, :, :, HD:HD + 1], 1.0)

        with tc.tile_pool(name="lnp", bufs=2) as lnp, \
             tc.tile_pool(name="lns", bufs=2) as lns, \
             tc.tile_pool(name="psSt", bufs=1, space="PSUM") as psSt, \
             tc.tile_pool(name="psK", bufs=2, space="PSUM") as psK, \
             tc.tile_pool(name="psQ", bufs=1, space="PSUM") as psQ, \
             tc.tile_pool(name="psV", bufs=2, space="PSUM") as psV:

            def load_xbT(ch):
                t = lnp.tile([P, KO, FC], bf16, tag="xbT")
                for ko in range(KO):
                    nc.sync.dma_start(
                        out=t[:, ko, :],
                        in_=io["xbT"].ap()[ko * P:(ko + 1) * P,
                                           ch * FC:(ch + 1) * FC])
                return t

            def emit_k(ct, ch):
                kp_ps = psK.tile([P, FC], f32, tag="k")
                cs = slice(ch * FC, (ch + 1) * FC)
                for kp in range(KP):
                    nc.tensor.matmul(
                        kp_ps, w_k8[:, ct, kp], xn8[:, 2 * kp:2 * kp + 2, cs],
                        start=(kp == 0), stop=(kp == KP - 1), perf_mode=DR)
                if ct % 4 == 3:
                    nc.scalar.activation(
                        out=kT[:, ct, cs], in_=kp_ps, func=AF.Identity,
                        bias=b_k8[:, ct:ct + 1], scale=0.125)
                else:
                    nc.vector.tensor_scalar(
                        out=kT[:, ct, cs], in0=kp_ps, scalar1=0.125,
                        scalar2=b_k8[:, ct:ct + 1], op0=ALU.mult, op1=ALU.add)

            pre = [load_xbT(0), load_xbT(1)]
            nc.sync.dma_start(out=w_v8, in_=io["w_v8"].ap())
            nc.sync.dma_start(out=w_k8, in_=io["w_k8"].ap())
            nc.sync.dma_start(out=b_k8, in_=io["b_k"].ap())
            nc.sync.dma_start(out=w_q8, in_=io["w_q8"].ap())
            nc.sync.dma_start(out=b_q8, in_=io["b_q"].ap())
            nc.sync.dma_start(out=ident, in_=io["ident"].ap())
            nc.sync.dma_start(out=w_p8, in_=io["w_p8"].ap())

            def emit_v(tt):
                vp = psV.tile([P, D], f32, tag="vp")
                for vc in range(2):
                    for kp in range(KP):
                        nc.tensor.matmul(
                            vp[:, vc * FC:(vc + 1) * FC],
                            xn8[:, 2 * kp:2 * kp + 2, tt * P:(tt + 1) * P],
                            w_v8[:, kp, :, vc * FC:(vc + 1) * FC],
                            start=(kp == 0), stop=(kp == KP - 1),
                            perf_mode=DR)
                nc.scalar.activation(
                    out=v_aug[:, tt, :, 0:HD],
                    in_=vp.rearrange("p (h d) -> p h d", h=NH),
                    func=AF.Identity, scale=0.125)

            for ch in range(NCH + 1):
                if ch < NCH:
                    sl = slice(ch * FC, (ch + 1) * FC)
                    xbT = pre[ch]
                    # stage 1a: stats + ln chain (ACT sqrt before stage-2 copies)
                    work = lnp.tile([P, KO, FC], bf16, tag="work")
                    nc.vector.tensor_mul(out=work, in0=xbT, in1=xbT)
                    st = psSt.tile([33, FC], f32, tag="st")
                    s_ps = st[0:1, :]
                    q_ps = st[32:33, :]
                    for ko in range(KO):
                        nc.tensor.matmul(s_ps, ones_invD, xbT[:, ko, :],
                                         start=(ko == 0), stop=(ko == KO - 1))
                    for ko in range(KO):
                        nc.tensor.matmul(q_ps, ones_invD, work[:, ko, :],
                                         start=(ko == 0), stop=(ko == KO - 1))
                    musq = lns.tile([1, FC], f32, tag="musq")
                    nc.scalar.square(out=musq, in_=s_ps)
                    ve = lns.tile([1, FC], f32, tag="ve")
                    nc.vector.tensor_tensor(out=ve, in0=q_ps, in1=musq,
                                            op=ALU.subtract)
                    std = lns.tile([1, FC], f32, tag="std")
                    nc.scalar.activation(out=std, in_=ve, func=AF.Sqrt,
                                         bias=eps1)
                    rstd = lns.tile([1, FC], bf16, tag="rstd")
                    with nc.allow_low_precision(reason="ln1 rstd bcast bf16"):
                        nc.vector.reciprocal(out=rstd, in_=std)
                    nmr = lns.tile([1, FC], bf16, tag="nmr")
                    nc.vector.tensor_mul(out=nmr, in0=s_ps, in1=rstd)
                    rn = lnp.tile([P, 2, FC], bf16, tag="rn")
                    nc.gpsimd.partition_broadcast(rn[:, 0, :], rstd)
                    nc.gpsimd.partition_broadcast(rn[:, 1, :], nmr)
                if ch >= 1:
                    # stage 2: V + K of the previous chunk
                    for tt in range((ch - 1) * 4, (ch - 1) * 4 + 4):
                        emit_v(tt)
                    for ct in range(KO):
                        emit_k(ct, ch - 1)
                if ch < NCH:
                    # stage 1b: normalize into fp8
                    for ko in range(KO):
                        nc.vector.tensor_mul(out=work[:, ko, :],
                                             in0=xbT[:, ko, :], in1=rn[:, 0, :])
                    for ko in range(KO):
                        nc.vector.tensor_tensor(
                            out=xn8[:, ko, sl], in0=work[:, ko, :],
                            in1=rn[:, 1, :], op=ALU.subtract)
                    if ch + 2 < NCH:
                        pre.append(load_xbT(ch + 2))
                if ch == NCH:
                    for ct in range(KO):
                        qp = psQ.tile([P, TC], f32, tag="q")
                        for s in range(NS):
                            qcol = (4 * s + 3) * P   # own tile at group pos 3
                            for kp in range(KP):
                                nc.tensor.matmul(
                                    qp[:, s * P:(s + 1) * P], w_q8[:, ct, kp],
                                    xn8[:, 2 * kp:2 * kp + 2, qcol:qcol + P],
                                    start=(kp == 0), stop=(kp == KP - 1),
                                    perf_mode=DR)
                        nc.vector.tensor_scalar(
                            out=qT[:, ct, :], in0=qp, scalar1=1.0 / 64.0,
                            scalar2=b_q8[:, ct:ct + 1], op0=ALU.mult,
                            op1=ALU.add)

        pA_cm.__exit__(None, None, None)

        # ---------- Phase C: attention, phase-D work interleaved per slot ----------
        pD_cm = tc.tile_pool(name="pD", bufs=1)
        pD = pD_cm.__enter__()
        x_own16 = pD.tile([P, NS, D], f32)
        mask_sb = pD.tile([P, NS, 4, P], bf16)
        nc.sync.dma_start(out=mask_sb, in_=io["mask"].ap())
        nc.sync.dma_start(
            out=x_own16,
            in_=bass.AP(tensor=io["x_own16"], offset=0,
                        ap=[[D, P], [D * P, NS], [1, D]]))
        mvs = {}
        with tc.tile_pool(name="attp", bufs=3) as attp, \
             tc.tile_pool(name="mvp", bufs=4) as mvp, \
             tc.tile_pool(name="dp", bufs=2) as dp, \
             tc.tile_pool(name="psC", bufs=3, space="PSUM") as psC, \
             tc.tile_pool(name="psAv", bufs=2, space="PSUM") as psAv:

            def emit_slot_d(s):
                """proj + residual for completed slot s (ln2 deferred)."""
                for ko in range(KO):
                    tp = psC.tile([P, P], bf16, tag="sc")
                    nc.tensor.transpose(
                        tp, attn_out[:, s, ko * P:(ko + 1) * P], identb)
                    nc.vector.tensor_copy(
                        out=attn_outT[:, ko, s * P:(s + 1) * P], in_=tp)
                for oc in range(2):
                    pj = psC.tile([P, 2, FC], f32, tag="sc")
                    for kp in range(KP):
                        nc.tensor.matmul(
                            pj[:, 0, :],
                            attn_outT[:, 2 * kp:2 * kp + 2, s * P:(s + 1) * P],
                            w_p8[:, kp, :, oc * FC:(oc + 1) * FC],
                            start=(kp == 0), stop=(kp == KP - 1), perf_mode=DR)
                    nc.vector.tensor_add(
                        out=X2[:, s, oc * FC:(oc + 1) * FC], in0=pj[:, 0, :],
                        in1=x_own16[:, s, oc * FC:(oc + 1) * FC])
                stats = dp.tile([P, 2, 6], f32, tag="st2")
                nc.vector.bn_stats(out=stats[:, 0, :], in_=X2[:, s, 0:FC])
                nc.vector.bn_stats(out=stats[:, 1, :], in_=X2[:, s, FC:D])
                mv = mvp.tile([P, 2], f32, tag="mv2")
                nc.vector.bn_aggr(out=mv, in_=stats)
                mvs[s] = mv

            def emit_slot_ln2(s):
                mv = mvs[s]
                std2 = dp.tile([P, 1], f32, tag="sd2")
                nc.scalar.activation(out=std2, in_=mv[:, 1:2], func=AF.Sqrt,
                                     bias=epsT)
                rstd2 = dp.tile([P, 1], f32, tag="rs2")
                nc.vector.reciprocal(out=rstd2, in_=std2)
                xn2 = dp.tile([P, D], f32r, tag="xn2")
                nc.vector.tensor_scalar(out=xn2, in0=X2[:, s, :],
                                        scalar1=mv[:, 0:1], scalar2=rstd2,
                                        op0=ALU.subtract, op1=ALU.mult)
                for ko in range(KO):
                    tp = psC.tile([P, P], f32r, tag="sc")
                    nc.tensor.transpose(tp, xn2[:, ko * P:(ko + 1) * P], ident)
                    nc.vector.tensor_copy(
                        out=xn2T[:, ko, s * P:(s + 1) * P], in_=tp)

            for s in range(NS):
                nseg = (CS[s] + 7) // 8
                for h in range(NH):
                    if h == 2 and s > 0:
                        emit_slot_d(s - 1)
                    hp = (h * HD) % P
                    hko = (h * HD) // P
                    attT = attp.tile([P, CS[s], P], bf16, tag=f"attT{s}")
                    qsl = qT[hp:hp + HD, hko, s * P:(s + 1) * P]
                    for seg in range(nseg):
                        n_kt = min(8, CS[s] - seg * 8)
                        sc = psC.tile([P, 2, FC], f32, tag="sc")
                        scv = sc.rearrange("p a b -> p (a b)").rearrange(
                            "p (k q) -> p k q", q=P)
                        for i in range(n_kt):
                            kt = seg * 8 + i
                            nc.tensor.matmul(
                                scv[:, i, :],
                                kT[hp:hp + HD, hko, kt * P:(kt + 1) * P],
                                qsl, start=True, stop=True)
                        nc.scalar.activation(
                            out=attT[:, seg * 8:seg * 8 + n_kt, :],
                            in_=scv[:, 0:n_kt, :], func=AF.Exp)
                    nc.vector.tensor_mul(
                        out=attT[:, CS[s] - 4:CS[s], :],
                        in0=attT[:, CS[s] - 4:CS[s], :],
                        in1=mask_sb[:, s])
                    av = psAv.tile([P, HD + 1], f32, tag="av")
                    for kt in range(CS[s]):
                        nc.tensor.matmul(av, attT[:, kt, :], v_aug[:, kt, h, :],
                                         start=(kt == 0), stop=(kt == CS[s] - 1))
                    recip = dp.tile([P, 1], f32, tag="recip")
                    nc.vector.reciprocal(out=recip, in_=av[:, HD:HD + 1])
                    nc.vector.tensor_scalar(
                        out=attn_out[:, s, h * HD:(h + 1) * HD],
                        in0=av[:, 0:HD], scalar1=recip, scalar2=None,
                        op0=ALU.mult)
            emit_slot_d(NS - 1)
            for s in range(NS):
                emit_slot_ln2(s)

        pD_cm.__exit__(None, None, None)
        pMain_cm.__exit__(None, None, None)

        # ---------- Phase E: fc1 -> gelu -> hT(fp8); fc2 + final residual ----------
        pE_cm = tc.tile_pool(name="pE", bufs=1)
        pE = pE_cm.__enter__()
        hT = pE.tile([P, NFT, TC], fp8)
        b2_rep16_t = pE.tile([P, D], f32)
        nc.sync.dma_start(out=b2_rep16_t, in_=io["b2_rep16"].ap())
        fc1_w8 = pE.tile([P, NFT, KP, 2, P], fp8)
        fc1_b = pE.tile([P, NFT], f32)
        nc.sync.dma_start(out=fc1_b, in_=io["fc1_b"].ap())
        # stream fc1 weights in 8 groups of 4 dff tiles so fc1 starts early
        for g in range(8):
            nc.sync.dma_start(out=fc1_w8[:, g * 4:(g + 1) * 4],
                              in_=io["fc1_w8"].ap()[:, g * 4:(g + 1) * 4])
        fc2_w8 = pE.tile([P, FP, 2, D], fp8)
        nc.sync.dma_start(out=fc2_w8, in_=io["fc2_w8"].ap())
        b2_rep16 = b2_rep16_t
        X2b = pE.tile([P, NS, D], f32)

        with tc.tile_pool(name="fe", bufs=3) as fe, \
             tc.tile_pool(name="psF", bufs=3, space="PSUM") as psF:
            for s in range(NS):
                nc.vector.tensor_add(out=X2b[:, s], in0=X2[:, s], in1=b2_rep16)
            for ft in range(NFT):
                f1 = psF.tile([P, TC], f32, tag="f1")
                for kp in range(KP):
                    nc.tensor.matmul(f1, fc1_w8[:, ft, kp],
                                     xn2T[:, 2 * kp:2 * kp + 2, :],
                                     start=(kp == 0), stop=(kp == KP - 1),
                                     perf_mode=DR)
                nc.scalar.activation(out=hT[:, ft, :], in_=f1, func=AF.Gelu,
                                     bias=fc1_b[:, ft:ft + 1], scale=0.125)
            for s in range(NS):
                for oc in range(2):
                    f2 = psF.tile([P, FC], f32, tag="f2")
                    for fp_ in range(FP):
                        nc.tensor.matmul(
                            f2, hT[:, 2 * fp_:2 * fp_ + 2, s * P:(s + 1) * P],
                            fc2_w8[:, fp_, :, oc * FC:(oc + 1) * FC],
                            start=(fp_ == 0), stop=(fp_ == FP - 1), perf_mode=DR)
                    t = fe.tile([P, FC], f32, tag="t")
                    nc.vector.tensor_add(out=t, in0=f2,
                                         in1=X2b[:, s, oc * FC:(oc + 1) * FC])
                    o = fe.tile([P, FC], f32, tag="o")
                    nc.vector.tensor_scalar(out=o, in0=t, scalar1=1.0 / 16.0,
                                            scalar2=None, op0=ALU.mult)
                    nc.sync.dma_start(
                        out=io["out"].ap()[s * P:(s + 1) * P,
                                           oc * FC:(oc + 1) * FC],
                        in_=o)
        pE_cm.__exit__(None, None, None)
        pLate_cm.__exit__(None, None, None)


def _stage_inputs(x, w_qkv, w_proj, ln1_w, ln1_b, ln2_w, ln2_b,
                  fc1_w, fc1_b, fc2_w, fc2_b):
    """Host-side sharding / ln folding / fp8 pre-scaling / tiling."""
    f = np.float32
    bf = ml_dtypes.bfloat16
    f8 = ml_dtypes.float8_e4m3
    x = np.asarray(x, f)
    w_qkv = np.asarray(w_qkv, f)
    ln1_w, ln1_b = np.asarray(ln1_w, f), np.asarray(ln1_b, f)
    ln2_w, ln2_b = np.asarray(ln2_w, f), np.asarray(ln2_b, f)
    fc1_wf, fc1_bf = np.asarray(fc1_w, f), np.asarray(fc1_b, f)
    fc2_wf, fc2_bf = np.asarray(fc2_w, f), np.asarray(fc2_b, f)
    w_projf = np.asarray(w_proj, f)

    wq_f = ln1_w[:, None] * w_qkv
    bq_f = ln1_b @ w_qkv
    scale = 1.0 / np.sqrt(HD)

    def pack_lhsT(w, mult):
        # [D, M] -> [P, M/P, KP, 2, P]; [p, ct, kp, i, m] = mult*w[(2kp+i)*P+p, ct*P+m]
        Din, M = w.shape
        t = (mult * w).reshape(KP, 2, P, M // P, P)
        return np.ascontiguousarray(t.transpose(2, 3, 0, 1, 4)).astype(f8)

    def pack_rhs(w, mult):
        # [D, N] -> [P, KP, 2, N]
        Din, N = w.shape
        t = (mult * w).reshape(KP, 2, P, N)
        return np.ascontiguousarray(t.transpose(2, 0, 1, 3)).astype(f8)

    w_q8 = pack_lhsT(wq_f[:, 0:D] * scale, 64.0)
    w_k8 = pack_lhsT(wq_f[:, D:2 * D], 8.0)
    w_v8 = pack_rhs(wq_f[:, 2 * D:3 * D], 8.0)
    w_p8 = pack_rhs(w_projf, 16.0)
    b_q_h = np.ascontiguousarray((bq_f[0:D] * scale).reshape(KO, P).T).astype(f)
    b_k_h = np.ascontiguousarray(bq_f[D:2 * D].reshape(KO, P).T).astype(f)
    b_v = bq_f[2 * D:3 * D]
    bvproj = b_v @ w_projf

    fc1s = ln2_w[:, None] * fc1_wf
    fc1_w8 = np.ascontiguousarray(
        (8.0 * fc1s).reshape(KP, 2, P, NFT, P).transpose(2, 3, 0, 1, 4)).astype(f8)
    fc1_b_h = np.ascontiguousarray(
        (ln2_b @ fc1_wf + fc1_bf).reshape(NFT, P).T).astype(f)
    fc2_w8 = np.ascontiguousarray(
        (16.0 * fc2_wf).reshape(FP, 2, P, D).transpose(2, 0, 1, 3)).astype(f8)
    b2_rep16 = np.ascontiguousarray(
        np.broadcast_to(16.0 * fc2_bf, (P, D))).astype(f)
    eye = np.eye(P, dtype=f)

    shared = {
        "w_k8": w_k8, "w_q8": w_q8, "w_v8": w_v8, "w_p8": w_p8,
        "fc1_w8": fc1_w8, "fc2_w8": fc2_w8,
        "b_k": b_k_h, "b_q": b_q_h, "fc1_b": fc1_b_h, "b2_rep16": b2_rep16,
        "ident": eye,
    }

    tri = np.tril(np.ones((P, P), np.float32)).T  # [p, q] = 1 iff p <= q
    in_maps = []
    for c in range(N_CORES):
        b = c // CPB
        j = c % CPB
        perm = []
        for g in range(NS):
            others = [4 * g + i for i in range(4) if i != j]
            perm += others + [4 * g + j]
        tok_perm = np.concatenate([np.arange(t * P, (t + 1) * P) for t in perm])
        xp = x[b][tok_perm]
        xbT_c = np.ascontiguousarray(xp.T).astype(bf)
        own_rows = np.concatenate(
            [np.arange((4 * s + j) * P, (4 * s + j + 1) * P) for s in range(NS)])
        x_own16_c = (16.0 * (x[b][own_rows] + bvproj)).astype(f)
        m = np.zeros((P, NS, 4, P), np.float32)
        for s in range(NS):
            for i in range(4):
                kt_abs = perm[4 * s + i]
                if kt_abs < 4 * s + j:
                    m[:, s, i, :] = 1.0
                elif kt_abs == 4 * s + j:
                    m[:, s, i, :] = tri
        mask_c = m.astype(bf)
        im = dict(shared)
        im.update({"xbT": xbT_c, "x_own16": x_own16_c, "mask": mask_c})
        in_maps.append(im)
    return in_maps


def kernel(**inputs) -> np.ndarray:
    if "nc" not in _CACHE:
        _CACHE["nc"] = build_nc()
    nc = _CACHE["nc"]
    in_maps = _stage_inputs(**inputs)
    res = run_bass_kernel_spmd(nc, in_maps, list(range(N_CORES)))
    out = np.empty((B, T, D), np.float32)
    for c in range(N_CORES):
        b = c // CPB
        j = c % CPB
        r = res.results[c]["out"]
        for s in range(NS):
            t_abs = 4 * s + j
            out[b, t_abs * P:(t_abs + 1) * P] = r[s * P:(s + 1) * P]
    return out


# revision 28
# speedup vs baseline: 1.0402x; 1.0402x over previous
"""Trainium2 Bass kernel for a pre-LN transformer block (B=2, T=2048, D=1024,
NH=16, HD=64, DFF=4096) on 8 NeuronCores.

Sharding: 4 cores per batch. Core j of a batch owns query tiles
{j, 4+j, 8+j, 12+j} (one per "slot" s=0..3), so every core does identical
causal-attention work: slot s attends to key tiles 0..4s+3 (40 score tiles
per head instead of 64). The host permutes each batch's tokens so the core's
own tile of group g sits at group position 4g+3; the key set covered by slot
s is unchanged, and causality within the last 4 key tiles of each slot is
applied via a small per-core data mask.

Precision: K/V/Q/proj/fc1/fc2 matmuls run in fp8e4m3 with DoubleRow perf
mode (two 128-deep contraction tiles per instruction). Weights are
pre-scaled by a power of two into fp8 normal range; the inverse scale folds
into the PSUM-evacuation copies. Scores and att@V stay bf16; fp32 PSUM
accumulation everywhere. ln1 runs channel-major via PE ones-matmul stats and
a rank-1 PSUM broadcast; ln2 token-major via bn_stats. The residual stream
is carried as 16*x so the proj/fc2 weight scale (16) cancels into a single
add per chunk.
"""

import sys

for _p in ("/opt/trn_rl_repo", "/root/.axon_site/_ro/trn_rl_repo"):
    if _p not in sys.path:
        sys.path.insert(0, _p)

import numpy as np
import ml_dtypes

import concourse.bass as bass
import concourse.tile as tile
from concourse import bacc, mybir
from concourse.bass_utils import run_bass_kernel_spmd

B = 2
T = 2048
D = 1024
NH = 16
HD = 64
DFF = 4 * D
EPS = 1e-5
P = 128
KO = D // P            # 8 contraction tiles over D
KP = KO // 2           # 4 DoubleRow pairs over D
N_CORES = 8
CPB = N_CORES // B     # cores per batch
TC = T // CPB          # 512 own tokens per core
NS = 4                 # query slots per core
NKT = T // P           # 16 key tiles
NFT = DFF // P         # 32 dff tiles
FP = NFT // 2          # 16 DoubleRow pairs over DFF
FC = 512
NCH = T // FC          # 4 token chunks for ln1
CS = [4, 8, 12, 16]    # key tiles per slot
TK = sum(CS)           # 40 score tiles per head

f32 = mybir.dt.float32
f32r = mybir.dt.float32r
bf16 = mybir.dt.bfloat16
fp8 = mybir.dt.float8e4
AF = mybir.ActivationFunctionType
ALU = mybir.AluOpType
DR = mybir.MatmulPerfMode.DoubleRow

_CACHE = {}


def build_nc():
    nc = bacc.Bacc("TRN2", target_bir_lowering=False)

    io = {}
    d = nc.declare_dram_parameter
    io["xbT"] = d("xbT", [D, T], bf16, isOutput=False)          # permuted, ch-major
    io["x_own16"] = d("x_own16", [TC, D], f32, isOutput=False)  # 16*(x_own+bvproj)
    io["w_k8"] = d("w_k8", [P, KO, KP, 2, P], fp8, isOutput=False)
    io["w_q8"] = d("w_q8", [P, KO, KP, 2, P], fp8, isOutput=False)
    io["w_v8"] = d("w_v8", [P, KP, 2, D], fp8, isOutput=False)
    io["w_p8"] = d("w_p8", [P, KP, 2, D], fp8, isOutput=False)
    io["fc1_w8"] = d("fc1_w8", [P, NFT, KP, 2, P], fp8, isOutput=False)
    io["fc2_w8"] = d("fc2_w8", [P, FP, 2, D], fp8, isOutput=False)
    io["b_k"] = d("b_k", [P, KO], f32, isOutput=False)
    io["b_q"] = d("b_q", [P, KO], f32, isOutput=False)
    io["fc1_b"] = d("fc1_b", [P, NFT], f32, isOutput=False)
    io["b2_rep16"] = d("b2_rep16", [P, D], f32, isOutput=False)
    io["mask"] = d("mask", [P, NS, 4, P], bf16, isOutput=False)
    io["ident"] = d("ident", [P, P], f32r, isOutput=False)
    io["out"] = d("out", [TC, D], f32, isOutput=True)

    with tile.TileContext(nc) as tc:
        _emit(nc, tc, io)
    nc.compile()
    return nc


def _emit(nc, tc, io):
    from contextlib import ExitStack

    with ExitStack() as ctx:
        singles = ctx.enter_context(tc.tile_pool(name="singles", bufs=1))

        ones_invD = singles.tile([P, 1], bf16)
        nc.vector.memset(ones_invD, 1.0 / D)
        eps1 = singles.tile([1, 1], f32)
        nc.vector.memset(eps1, EPS)
        epsT = singles.tile([P, 1], f32)
        nc.vector.memset(epsT, EPS * 256.0)          # ln2 runs on 16*x
        ident = singles.tile([P, P], f32r)
        identb = singles.tile([P, P], bf16)
        nc.vector.tensor_copy(out=identb, in_=ident)

        # ---- tiles that live into phases D/E (must be below pMain on the
        # pool stack, so allocated up front) ----
        pLate_cm = tc.tile_pool(name="pLate", bufs=1)
        pLate = pLate_cm.__enter__()
        attn_out = pLate.tile([P, NS, D], bf16)
        attn_outT = pLate.tile([P, KO, TC], fp8)
        X2 = pLate.tile([P, NS, D], f32)
        xn2T = pLate.tile([P, KO, TC], fp8)
        w_p8 = pLate.tile([P, KP, 2, D], fp8)

        # ---- persistent big tiles (live through phase C) ----
        pMain_cm = tc.tile_pool(name="pMain", bufs=1)
        pMain = pMain_cm.__enter__()
        kT = pMain.tile([P, KO, T], bf16)
        v_aug = pMain.tile([P, NKT, NH, HD + 1], bf16)
        qT = pMain.tile([P, KO, TC], bf16)

        # ---------- Phase A: ln1 channel-major, V interleaved ----------
        pA_cm = tc.tile_pool(name="pA", bufs=1)
        pA = pA_cm.__enter__()
        xn8 = pA.tile([P, KO, T], fp8)
        w_v8 = pA.tile([P, KP, 2, D], fp8)
        w_k8 = pA.tile([P, KO, KP, 2, P], fp8)
        w_q8 = pA.tile([P, KO, KP, 2, P], fp8)
        b_k8 = pA.tile([P, KO], f32)
        b_q8 = pA.tile([P, KO], f32)
        nc.vector.memset(v_aug[:, :, :, HD:HD + 1], 1.0)

        with tc.tile_pool(name="lnp", bufs=2) as lnp, \
             tc.tile_pool(name="lns", bufs=2) as lns, \
             tc.tile_pool(name="psSt", bufs=1, space="PSUM") as psSt, \
             tc.tile_pool(name="psK", bufs=2, space="PSUM") as psK, \
             tc.tile_pool(name="psQ", bufs=1, space="PSUM") as psQ, \
             tc.tile_pool(name="psV", bufs=2, space="PSUM") as psV:

            def load_xbT(ch):
                t = lnp.tile([P, KO, FC], bf16, tag="xbT")
                for ko in range(KO):
                    nc.sync.dma_start(
                        out=t[:, ko, :],
                        in_=io["xbT"].ap()[ko * P:(ko + 1) * P,
                                           ch * FC:(ch + 1) * FC])
                return t

            def emit_k(ct, ch):
                kp_ps = psK.tile([P, FC], f32, tag="k")
                cs = slice(ch * FC, (ch + 1) * FC)
                for kp in range(KP):
                    nc.tensor.matmul(
                        kp_ps, w_k8[:, ct, kp], xn8[:, 2 * kp:2 * kp + 2, cs],
                        start=(kp == 0), stop=(kp == KP - 1), perf_mode=DR)
                nc.scalar.activation(
                    out=kT[:, ct, cs], in_=kp_ps, func=AF.Identity,
                    bias=b_k8[:, ct:ct + 1], scale=0.125)

            pre = [load_xbT(0), load_xbT(1)]
            nc.sync.dma_start(out=w_v8, in_=io["w_v8"].ap())
            nc.sync.dma_start(out=w_k8, in_=io["w_k8"].ap())
            nc.sync.dma_start(out=b_k8, in_=io["b_k"].ap())
            nc.sync.dma_start(out=w_q8, in_=io["w_q8"].ap())
            nc.sync.dma_start(out=b_q8, in_=io["b_q"].ap())
            nc.sync.dma_start(out=ident, in_=io["ident"].ap())
            nc.sync.dma_start(out=w_p8, in_=io["w_p8"].ap())

            def emit_v(tt):
                vp = psV.tile([P, D], f32, tag="vp")
                for vc in range(2):
                    for kp in range(KP):
                        nc.tensor.matmul(
                            vp[:, vc * FC:(vc + 1) * FC],
                            xn8[:, 2 * kp:2 * kp + 2, tt * P:(tt + 1) * P],
                            w_v8[:, kp, :, vc * FC:(vc + 1) * FC],
                            start=(kp == 0), stop=(kp == KP - 1),
                            perf_mode=DR)
                nc.scalar.activation(
                    out=v_aug[:, tt, :, 0:HD],
                    in_=vp.rearrange("p (h d) -> p h d", h=NH),
                    func=AF.Identity, scale=0.125)

            for ch in range(NCH + 1):
                if ch < NCH:
                    sl = slice(ch * FC, (ch + 1) * FC)
                    xbT = pre[ch]
                    # stage 1a: stats + ln chain (ACT sqrt before stage-2 copies)
                    work = lnp.tile([P, KO, FC], bf16, tag="work")
                    nc.vector.tensor_mul(out=work, in0=xbT, in1=xbT)
                    st = psSt.tile([33, FC], f32, tag="st")
                    s_ps = st[0:1, :]
                    q_ps = st[32:33, :]
                    for ko in range(KO):
                        nc.tensor.matmul(s_ps, ones_invD, xbT[:, ko, :],
                                         start=(ko == 0), stop=(ko == KO - 1))
                    for ko in range(KO):
                        nc.tensor.matmul(q_ps, ones_invD, work[:, ko, :],
                                         start=(ko == 0), stop=(ko == KO - 1))
                    musq = lns.tile([1, FC], f32, tag="musq")
                    nc.scalar.square(out=musq, in_=s_ps)
                    ve = lns.tile([1, FC], f32, tag="ve")
                    nc.vector.tensor_tensor(out=ve, in0=q_ps, in1=musq,
                                            op=ALU.subtract)
                    std = lns.tile([1, FC], f32, tag="std")
                    nc.scalar.activation(out=std, in_=ve, func=AF.Sqrt,
                                         bias=eps1)
                    rstd = lns.tile([1, FC], bf16, tag="rstd")
                    with nc.allow_low_precision(reason="ln1 rstd bcast bf16"):
                        nc.vector.reciprocal(out=rstd, in_=std)
                    nmr = lns.tile([1, FC], bf16, tag="nmr")
                    nc.vector.tensor_mul(out=nmr, in0=s_ps, in1=rstd)
                    rn = lnp.tile([P, 2, FC], bf16, tag="rn")
                    nc.gpsimd.partition_broadcast(rn[:, 0, :], rstd)
                    nc.gpsimd.partition_broadcast(rn[:, 1, :], nmr)
                if ch >= 1:
                    # stage 2: V + K of the previous chunk
                    for tt in range((ch - 1) * 4, (ch - 1) * 4 + 4):
                        emit_v(tt)
                    for ct in range(KO):
                        emit_k(ct, ch - 1)
                if ch < NCH:
                    # stage 1b: normalize into fp8
                    for ko in range(KO):
                        nc.vector.tensor_mul(out=work[:, ko, :],
                                             in0=xbT[:, ko, :], in1=rn[:, 0, :])
                    for ko in range(KO):
                        nc.vector.tensor_tensor(
                            out=xn8[:, ko, sl], in0=work[:, ko, :],
                            in1=rn[:, 1, :], op=ALU.subtract)
                    if ch + 2 < NCH:
                        pre.append(load_xbT(ch + 2))
                if ch == NCH:
                    for ct in range(KO):
                        qp = psQ.tile([P, TC], f32, tag="q")
                        for s in range(NS):
                            qcol = (4 * s + 3) * P   # own tile at group pos 3
                            for kp in range(KP):
                                nc.tensor.matmul(
                                    qp[:, s * P:(s + 1) * P], w_q8[:, ct, kp],
                                    xn8[:, 2 * kp:2 * kp + 2, qcol:qcol + P],
                                    start=(kp == 0), stop=(kp == KP - 1),
                                    perf_mode=DR)
                        nc.vector.tensor_scalar(
                            out=qT[:, ct, :], in0=qp, scalar1=1.0 / 64.0,
                            scalar2=b_q8[:, ct:ct + 1], op0=ALU.mult,
                            op1=ALU.add)

        pA_cm.__exit__(None, None, None)

        # ---------- Phase C: attention, phase-D work interleaved per slot ----------
        pD_cm = tc.tile_pool(name="pD", bufs=1)
        pD = pD_cm.__enter__()
        x_own16 = pD.tile([P, NS, D], f32)
        mask_sb = pD.tile([P, NS, 4, P], bf16)
        nc.sync.dma_start(out=mask_sb, in_=io["mask"].ap())
        nc.sync.dma_start(
            out=x_own16,
            in_=bass.AP(tensor=io["x_own16"], offset=0,
                        ap=[[D, P], [D * P, NS], [1, D]]))
        mvs = {}
        with tc.tile_pool(name="attp", bufs=3) as attp, \
             tc.tile_pool(name="mvp", bufs=4) as mvp, \
             tc.tile_pool(name="dp", bufs=2) as dp, \
             tc.tile_pool(name="psC", bufs=3, space="PSUM") as psC, \
             tc.tile_pool(name="psAv", bufs=2, space="PSUM") as psAv:

            def emit_slot_d(s):
                """proj + residual for completed slot s (ln2 deferred)."""
                for ko in range(KO):
                    tp = psC.tile([P, P], bf16, tag="sc")
                    nc.tensor.transpose(
                        tp, attn_out[:, s, ko * P:(ko + 1) * P], identb)
                    nc.vector.tensor_copy(
                        out=attn_outT[:, ko, s * P:(s + 1) * P], in_=tp)
                for oc in range(2):
                    pj = psC.tile([P, 2, FC], f32, tag="sc")
                    for kp in range(KP):
                        nc.tensor.matmul(
                            pj[:, 0, :],
                            attn_outT[:, 2 * kp:2 * kp + 2, s * P:(s + 1) * P],
                            w_p8[:, kp, :, oc * FC:(oc + 1) * FC],
                            start=(kp == 0), stop=(kp == KP - 1), perf_mode=DR)
                    nc.vector.tensor_add(
                        out=X2[:, s, oc * FC:(oc + 1) * FC], in0=pj[:, 0, :],
                        in1=x_own16[:, s, oc * FC:(oc + 1) * FC])
                stats = dp.tile([P, 2, 6], f32, tag="st2")
                nc.vector.bn_stats(out=stats[:, 0, :], in_=X2[:, s, 0:FC])
                nc.vector.bn_stats(out=stats[:, 1, :], in_=X2[:, s, FC:D])
                mv = mvp.tile([P, 2], f32, tag="mv2")
                nc.vector.bn_aggr(out=mv, in_=stats)
                mvs[s] = mv

            def emit_slot_ln2(s):
                mv = mvs[s]
                std2 = dp.tile([P, 1], f32, tag="sd2")
                nc.scalar.activation(out=std2, in_=mv[:, 1:2], func=AF.Sqrt,
                                     bias=epsT)
                rstd2 = dp.tile([P, 1], f32, tag="rs2")
                nc.vector.reciprocal(out=rstd2, in_=std2)
                xn2 = dp.tile([P, D], f32r, tag="xn2")
                nc.vector.tensor_scalar(out=xn2, in0=X2[:, s, :],
                                        scalar1=mv[:, 0:1], scalar2=rstd2,
                                        op0=ALU.subtract, op1=ALU.mult)
                for ko in range(KO):
                    tp = psC.tile([P, P], f32r, tag="sc")
                    nc.tensor.transpose(tp, xn2[:, ko * P:(ko + 1) * P], ident)
                    if ko % 2 == 0:
                        nc.scalar.copy(
                            out=xn2T[:, ko, s * P:(s + 1) * P], in_=tp)
                    else:
                        nc.vector.tensor_copy(
                            out=xn2T[:, ko, s * P:(s + 1) * P], in_=tp)

            for s in range(NS):
                nseg = (CS[s] + 7) // 8
                for h in range(NH):
                    if h == 2 and s > 0:
                        emit_slot_d(s - 1)
                    hp = (h * HD) % P
                    hko = (h * HD) // P
                    attT = attp.tile([P, CS[s], P], bf16, tag=f"attT{s}")
                    qsl = qT[hp:hp + HD, hko, s * P:(s + 1) * P]
                    for seg in range(nseg):
                        n_kt = min(8, CS[s] - seg * 8)
                        sc = psC.tile([P, 2, FC], f32, tag="sc")
                        scv = sc.rearrange("p a b -> p (a b)").rearrange(
                            "p (k q) -> p k q", q=P)
                        for i in range(n_kt):
                            kt = seg * 8 + i
                            nc.tensor.matmul(
                                scv[:, i, :],
                                kT[hp:hp + HD, hko, kt * P:(kt + 1) * P],
                                qsl, start=True, stop=True)
                        nc.scalar.activation(
                            out=attT[:, seg * 8:seg * 8 + n_kt, :],
                            in_=scv[:, 0:n_kt, :], func=AF.Exp)
                    nc.vector.tensor_mul(
                        out=attT[:, CS[s] - 4:CS[s], :],
                        in0=attT[:, CS[s] - 4:CS[s], :],
                        in1=mask_sb[:, s])
                    av = psAv.tile([P, HD + 1], f32, tag="av")
                    for kt in range(CS[s]):
                        nc.tensor.matmul(av, attT[:, kt, :], v_aug[:, kt, h, :],
                                         start=(kt == 0), stop=(kt == CS[s] - 1))
                    recip = dp.tile([P, 1], f32, tag="recip")
                    nc.vector.reciprocal(out=recip, in_=av[:, HD:HD + 1])
                    nc.vector.tensor_scalar(
                        out=attn_out[:, s, h * HD:(h + 1) * HD],
                        in0=av[:, 0:HD], scalar1=recip, scalar2=None,
                        op0=ALU.mult)
            emit_slot_d(NS - 1)
            for s in range(NS):
                emit_slot_ln2(s)

        pD_cm.__exit__(None, None, None)
        pMain_cm.__exit__(None, None, None)

        # ---------- Phase E: fc1 -> gelu -> hT(fp8); fc2 + final residual ----------
        pE_cm = tc.tile_pool(name="pE", bufs=1)
        pE = pE_cm.__enter__()
        hT = pE.tile([P, NFT, TC], fp8)
        b2_rep16_t = pE.tile([P, D], f32)
        nc.sync.dma_start(out=b2_rep16_t, in_=io["b2_rep16"].ap())
        fc1_w8 = pE.tile([P, NFT, KP, 2, P], fp8)
        fc1_b = pE.tile([P, NFT], f32)
        nc.sync.dma_start(out=fc1_b, in_=io["fc1_b"].ap())
        # stream fc1 weights in 8 groups of 4 dff tiles so fc1 starts early
        for g in range(8):
            nc.sync.dma_start(out=fc1_w8[:, g * 4:(g + 1) * 4],
                              in_=io["fc1_w8"].ap()[:, g * 4:(g + 1) * 4])
        fc2_w8 = pE.tile([P, FP, 2, D], fp8)
        nc.sync.dma_start(out=fc2_w8, in_=io["fc2_w8"].ap())
        b2_rep16 = b2_rep16_t
        X2b = pE.tile([P, NS, D], f32)

        with tc.tile_pool(name="fe", bufs=3) as fe, \
             tc.tile_pool(name="psF", bufs=3, space="PSUM") as psF:
            for s in range(NS):
                nc.vector.tensor_add(out=X2b[:, s], in0=X2[:, s], in1=b2_rep16)
            for ft in range(NFT):
                f1 = psF.tile([P, TC], f32, tag="f1")
                for kp in range(KP):
                    nc.tensor.matmul(f1, fc1_w8[:, ft, kp],
                                     xn2T[:, 2 * kp:2 * kp + 2, :],
                                     start=(kp == 0), stop=(kp == KP - 1),
                                     perf_mode=DR)
                nc.scalar.activation(out=hT[:, ft, :], in_=f1, func=AF.Gelu,
                                     bias=fc1_b[:, ft:ft + 1], scale=0.125)
            for s in range(NS):
                for oc in range(2):
                    f2 = psF.tile([P, FC], f32, tag="f2")
                    for fp_ in range(FP):
                        nc.tensor.matmul(
                            f2, hT[:, 2 * fp_:2 * fp_ + 2, s * P:(s + 1) * P],
                            fc2_w8[:, fp_, :, oc * FC:(oc + 1) * FC],
                            start=(fp_ == 0), stop=(fp_ == FP - 1), perf_mode=DR)
                    t = fe.tile([P, FC], f32, tag="t")
                    nc.vector.tensor_add(out=t, in0=f2,
                                         in1=X2b[:, s, oc * FC:(oc + 1) * FC])
                    o = fe.tile([P, FC], f32, tag="o")
                    nc.vector.tensor_scalar(out=o, in0=t, scalar1=1.0 / 16.0,
                                            scalar2=None, op0=ALU.mult)
                    nc.sync.dma_start(
                        out=io["out"].ap()[s * P:(s + 1) * P,
                                           oc * FC:(oc + 1) * FC],
                        in_=o)
        pE_cm.__exit__(None, None, None)
        pLate_cm.__exit__(None, None, None)


def _stage_inputs(x, w_qkv, w_proj, ln1_w, ln1_b, ln2_w, ln2_b,
                  fc1_w, fc1_b, fc2_w, fc2_b):
    """Host-side sharding / ln folding / fp8 pre-scaling / tiling."""
    f = np.float32
    bf = ml_dtypes.bfloat16
    f8 = ml_dtypes.float8_e4m3
    x = np.asarray(x, f)
    w_qkv = np.asarray(w_qkv, f)
    ln1_w, ln1_b = np.asarray(ln1_w, f), np.asarray(ln1_b, f)
    ln2_w, ln2_b = np.asarray(ln2_w, f), np.asarray(ln2_b, f)
    fc1_wf, fc1_bf = np.asarray(fc1_w, f), np.asarray(fc1_b, f)
    fc2_wf, fc2_bf = np.asarray(fc2_w, f), np.asarray(fc2_b, f)
    w_projf = np.asarray(w_proj, f)

    wq_f = ln1_w[:, None] * w_qkv
    bq_f = ln1_b @ w_qkv
    scale = 1.0 / np.sqrt(HD)

    def pack_lhsT(w, mult):
        # [D, M] -> [P, M/P, KP, 2, P]; [p, ct, kp, i, m] = mult*w[(2kp+i)*P+p, ct*P+m]
        Din, M = w.shape
        t = (mult * w).reshape(KP, 2, P, M // P, P)
        return np.ascontiguousarray(t.transpose(2, 3, 0, 1, 4)).astype(f8)

    def pack_rhs(w, mult):
        # [D, N] -> [P, KP, 2, N]
        Din, N = w.shape
        t = (mult * w).reshape(KP, 2, P, N)
        return np.ascontiguousarray(t.transpose(2, 0, 1, 3)).astype(f8)

    w_q8 = pack_lhsT(wq_f[:, 0:D] * scale, 64.0)
    w_k8 = pack_lhsT(wq_f[:, D:2 * D], 8.0)
    w_v8 = pack_rhs(wq_f[:, 2 * D:3 * D], 8.0)
    w_p8 = pack_rhs(w_projf, 16.0)
    b_q_h = np.ascontiguousarray((bq_f[0:D] * scale).reshape(KO, P).T).astype(f)
    b_k_h = np.ascontiguousarray(bq_f[D:2 * D].reshape(KO, P).T).astype(f)
    b_v = bq_f[2 * D:3 * D]
    bvproj = b_v @ w_projf

    fc1s = ln2_w[:, None] * fc1_wf
    fc1_w8 = np.ascontiguousarray(
        (8.0 * fc1s).reshape(KP, 2, P, NFT, P).transpose(2, 3, 0, 1, 4)).astype(f8)
    fc1_b_h = np.ascontiguousarray(
        (ln2_b @ fc1_wf + fc1_bf).reshape(NFT, P).T).astype(f)
    fc2_w8 = np.ascontiguousarray(
        (16.0 * fc2_wf).reshape(FP, 2, P, D).transpose(2, 0, 1, 3)).astype(f8)
    b2_rep16 = np.ascontiguousarray(
        np.broadcast_to(16.0 * fc2_bf, (P, D))).astype(f)
    eye = np.eye(P, dtype=f)

    shared = {
        "w_k8": w_k8, "w_q8": w_q8, "w_v8": w_v8, "w_p8": w_p8,
        "fc1_w8": fc1_w8, "fc2_w8": fc2_w8,
        "b_k": b_k_h, "b_q": b_q_h, "fc1_b": fc1_b_h, "b2_rep16": b2_rep16,
        "ident": eye,
    }

    tri = np.tril(np.ones((P, P), np.float32)).T  # [p, q] = 1 iff p <= q
    in_maps = []
    for c in range(N_CORES):
        b = c // CPB
        j = c % CPB
        perm = []
        for g in range(NS):
            others = [4 * g + i for i in range(4) if i != j]
            perm += others + [4 * g + j]
        tok_perm = np.concatenate([np.arange(t * P, (t + 1) * P) for t in perm])
        xp = x[b][tok_perm]
        xbT_c = np.ascontiguousarray(xp.T).astype(bf)
        own_rows = np.concatenate(
            [np.arange((4 * s + j) * P, (4 * s + j + 1) * P) for s in range(NS)])
        x_own16_c = (16.0 * (x[b][own_rows] + bvproj)).astype(f)
        m = np.zeros((P, NS, 4, P), np.float32)
        for s in range(NS):
            for i in range(4):
                kt_abs = perm[4 * s + i]
                if kt_abs < 4 * s + j:
                    m[:, s, i, :] = 1.0
                elif kt_abs == 4 * s + j:
                    m[:, s, i, :] = tri
        mask_c = m.astype(bf)
        im = dict(shared)
        im.update({"xbT": xbT_c, "x_own16": x_own16_c, "mask": mask_c})
        in_maps.append(im)
    return in_maps


def kernel(**inputs) -> np.ndarray:
    if "nc" not in _CACHE:
        _CACHE["nc"] = build_nc()
    nc = _CACHE["nc"]
    in_maps = _stage_inputs(**inputs)
    res = run_bass_kernel_spmd(nc, in_maps, list(range(N_CORES)))
    out = np.empty((B, T, D), np.float32)
    for c in range(N_CORES):
        b = c // CPB
        j = c % CPB
        r = res.results[c]["out"]
        for s in range(NS):
            t_abs = 4 * s + j
            out[b, t_abs * P:(t_abs + 1) * P] = r[s * P:(s + 1) * P]
    return out
